# revision 20
# baseline (speedup 1.0000x reference)
"""Trainium2 Bass kernel: per-sample hypernetwork depthwise 3x3 conv.

Reference computation (per batch b):
    W_dw[b] = (z[b] @ W_lin.T).reshape(OUT_C, 1, 3, 3)
    y[b]    = depthwise_conv2d(x[b], W_dw[b], padding=1)

Sharding: data-parallel over batch across 8 NeuronCores (2 batches/core),
W_lin replicated. Each core computes its own W_dw on-device.

Per-core design:
  - channels (256) -> 2 groups of 128 on SBUF partitions
  - image split into 32-row bands; each band is DMA'd as one fully
    contiguous 17.4KB-per-partition transfer into an UNPADDED flat tile
    (rows r0-1..r0+32 back to back, one pad element at each end).
    Width-edge taps wrap into the neighboring row; those wrong
    contributions are subtracted afterwards by small "correction" ops
    with negated weights reading the exact same wrapped positions.
  - 9 conv taps split between engines per 16-row PSUM group, alternating
    7-PE/2-DVE and 6-PE/3-DVE:
      * PE taps: diagonal-weight matmuls (float32r) accumulating in PSUM
      * DVE taps: fused scalar_tensor_tensor FMAs; the first also drains
        PSUM
  - ScalarE (ACT) makes the f32r rounded copy of each band and issues the
    output DMAs (separate HWDGE ring from the input DMAs on SyncE)
  - GpSimd only does the small halo memsets
  - W_dw computed on-device by 18 small fp32 matmuls from a host-side
    re-layout of W_lin (pure permutation/transpose, no host math)
"""

import os
import sys

for _p in ("/opt/trn_rl_repo", "/root/.axon_site", "/root/.axon_site/_ro/trn_rl_repo",
           "/root/.axon_site/_ro/pypackages"):
    if os.path.isdir(_p) and _p not in sys.path:
        sys.path.append(_p)

import numpy as np

import concourse.bass as bass
import concourse.tile as tile
from concourse import bacc, mybir
from concourse import bass_utils
from concourse.alu_op_type import AluOpType

F32 = mybir.dt.float32
F32R = mybir.dt.float32r

# problem constants (hardcoded per contract)
B, OUT_C, H, W = 16, 256, 128, 128
K, Z_DIM = 3, 64
N_CORES = 8
B_PER = B // N_CORES          # 2 batches per core
G = OUT_C // 128              # 2 channel groups of 128

# tuning knobs
PE_TAP_PATTERN = (7, 6)       # taps on PE per psum-group, alternating
ROWS_BAND = 32
ROWS_GROUP = 16
MAX_PE_TAPS = max(PE_TAP_PATTERN)

TAPS = [(dy, dx) for dy in range(3) for dx in range(3)]


def build_nc(pe_pattern=PE_TAP_PATTERN, rows_band=ROWS_BAND, rows_group=ROWS_GROUP,
             b_per=B_PER, h=H):
    """Build the per-core Bass program. Returns compiled Bacc object."""
    n_bands = h // rows_band
    grp_per_band = rows_band // rows_group
    banks_per_grp = rows_group * W // 512
    grp_free = rows_group * W
    band_free = rows_band * W
    pad_rows = rows_band + 2
    # data at flat [1 + t*W, 1 + (t+1)*W); leading pad at 0, trailing wrap pad
    # at pad_rows*W + 1; extra W slack so stride-W correction APs stay in range
    flat_n = pad_rows * W + 2 + W
    max_pe = 9  # final band runs all taps on PE

    nc = bacc.Bacc("TRN2", target_bir_lowering=False, debug=False)

    x_d = nc.dram_tensor("x", [b_per, OUT_C, h, W], F32, kind="ExternalInput")
    zt_d = nc.dram_tensor("zT", [Z_DIM, b_per], F32, kind="ExternalInput")
    wlt_d = nc.dram_tensor("wlt", [Z_DIM, OUT_C * K * K], F32, kind="ExternalInput")
    ident_d = nc.dram_tensor("ident", [128, 128], F32, kind="ExternalInput")
    zeros_d = nc.dram_tensor("zeros", [128, W], F32, kind="ExternalInput")
    y_d = nc.dram_tensor("y", [b_per, OUT_C, h, W], F32, kind="ExternalOutput")

    n_chunks = OUT_C * K * K // 128          # 18
    wd_cols = K * K * G * b_per              # 36, col = (g*9 + t)*b_per + b

    with tile.TileContext(nc) as tc:
        with tc.tile_pool(name="wconst", bufs=1) as wpool:
            ident = wpool.tile([128, 128], F32)
            nc.sync.dma_start(ident[:], ident_d.ap()[:, :])
            wlt = wpool.tile([Z_DIM, OUT_C * K * K], F32)
            half = OUT_C * K * K // 2
            nc.sync.dma_start(wlt[:, 0:half], wlt_d.ap()[:, 0:half])
            nc.sync.dma_start(wlt[:, half:], wlt_d.ap()[:, half:])
            zt = wpool.tile([Z_DIM, b_per], F32)
            nc.sync.dma_start(zt[:], zt_d.ap()[:, :])

            wd = wpool.tile([128, wd_cols], F32)
            with tc.tile_pool(name="wpsum", bufs=2, space="PSUM") as wps:
                for j in range(n_chunks):
                    ps = wps.tile([128, b_per], F32)
                    nc.tensor.matmul(ps[:], wlt[:, 128 * j:128 * (j + 1)], zt[:],
                                     start=True, stop=True)
                    nc.scalar.copy(wd[:, b_per * j:b_per * (j + 1)], ps[:])

            # dummy matmuls keep the PE busy through the diag-build gap so
            # the HAM clock gate is warm when the tap matmuls start
            with tc.tile_pool(name="warm", bufs=1, space="PSUM") as warmp:
                wps_t = warmp.tile([128, 128], F32)
                for _ in range(8):
                    nc.tensor.matmul(wps_t[:], ident[:], ident[:],
                                     start=True, stop=True)

            # negated weights for the wrap corrections
            wdn = wpool.tile([128, wd_cols], F32)
            nc.scalar.mul(wdn[:], wd[:], -1.0)

            # diagonal weight matrices (f32r) for all taps that may run on PE
            diags = {}
            for b in range(b_per):
                for g in range(G):
                    for ti in range(max_pe):
                        col = (g * K * K + ti) * b_per + b
                        dtile = wpool.tile([128, 128], F32R, tag=f"diag_{b}_{g}_{ti}")
                        nc.scalar.mul(dtile[:], ident[:], wd[:, col:col + 1])
                        diags[(b, g, ti)] = dtile

            with tc.tile_pool(name="xband", bufs=6) as xpool, \
                 tc.tile_pool(name="oband", bufs=4) as opool, \
                 tc.tile_pool(name="psum", bufs=2, space="PSUM") as pspool:
                gidx = 0
                band_no = 0
                for b in range(b_per):
                    for g in range(G):
                        prev_xt = None
                        for band in range(n_bands):
                            r0 = band * rows_band
                            # rows r0-1, r0 come from the previous band's tile
                            # (SBUF->SBUF); only new rows come from HBM
                            lo = 0 if band == 0 else r0 + 1
                            hi = min(r0 + rows_band + 1, h)
                            # tile row t holds image row r0-1+t at flat
                            # [1 + t*W, 1 + (t+1)*W)
                            xt = xpool.tile([128, flat_n], F32)
                            xtr = xt[:].bitcast(F32R)
                            # zero wrap-pad elements once per pool buffer; later
                            # reuses hold stale-but-finite values that cancel
                            # exactly against the corrections
                            if band_no < 6:
                                for off in (0, pad_rows * W + 1):
                                    nc.vector.tensor_scalar(
                                        out=xtr[:, off:off + 1], in0=ident[:, 0:1],
                                        scalar1=0.0, scalar2=None, op0=AluOpType.mult)
                            band_no += 1
                            if r0 == 0:
                                nc.scalar.dma_start(xtr[:, 1:1 + W],
                                                    zeros_d.ap()[:, :].bitcast(F32R))
                            if r0 + rows_band == h:
                                nc.scalar.dma_start(
                                    xtr[:, 1 + (pad_rows - 1) * W:1 + pad_rows * W],
                                    zeros_d.ap()[:, :].bitcast(F32R))
                            # dest AP typed f32r: PE reads it as f32r (the HW
                            # rounds on read); DVE reads the same bits as f32
                            if band > 0:
                                nc.sync.dma_start(
                                    xtr[:, 1:1 + 2 * W],
                                    prev_xt[:, 1 + (rows_band) * W:
                                            1 + (rows_band + 2) * W].bitcast(F32R))
                            dst0 = 1 + (lo - (r0 - 1)) * W
                            nc.sync.dma_start(
                                xt[:, dst0:dst0 + (hi - lo) * W].bitcast(F32R),
                                x_d.ap()[b, 128 * g:128 * (g + 1), lo:hi, :]
                                .bitcast(F32R))
                            prev_xt = xt[:]
                            xr = xt[:].bitcast(F32R)

                            ot = opool.tile([128, band_free], F32)
                            last_band = (b == b_per - 1 and g == G - 1
                                         and band == n_bands - 1)
                            for grp in range(grp_per_band):
                                gr0 = grp * rows_group
                                n_pe = 9 if last_band else \
                                    pe_pattern[gidx % len(pe_pattern)]
                                gidx += 1
                                pe_taps = TAPS[:n_pe]
                                dve_taps = TAPS[n_pe:]

                                ps = pspool.tile([128, grp_free], F32)
                                rows_bank = 512 // W
                                for ti in range(n_pe):
                                    dy, dx = pe_taps[ti]
                                    for bank in range(banks_per_grp):
                                        s = (gr0 + bank * rows_bank + dy) * W + dx
                                        nc.tensor.matmul(
                                            ps[:, 512 * bank:512 * (bank + 1)],
                                            diags[(b, g, ti)][:],
                                            xr[:, s:s + 512],
                                            start=(ti == 0),
                                            stop=(ti == n_pe - 1))

                                og = ot[:, gr0 * W:gr0 * W + grp_free]
                                acc = ps[:]
                                for k, (dy, dx) in enumerate(dve_taps):
                                    ti = n_pe + k
                                    col = (g * K * K + ti) * b_per + b
                                    s = (gr0 + dy) * W + dx
                                    nc.vector.scalar_tensor_tensor(
                                        out=og, in0=xt[:, s:s + grp_free],
                                        scalar=wd[:, col:col + 1], in1=acc,
                                        op0=AluOpType.mult, op1=AluOpType.add)
                                    acc = og
                                if not dve_taps:
                                    nc.vector.tensor_copy(og, ps[:])

                            # width-edge wrap corrections over the whole band:
                            # og[r, 0]   -= w[dy,0] * flat[(r+dy)*W]      (left)
                            # og[r, W-1] -= w[dy,2] * flat[(r+dy+1)*W+1]  (right)
                            otv = ot[:].rearrange("p (r c) -> p r c", c=W)
                            for dy in range(3):
                                for dx, (off, oc) in (
                                        (0, (dy * W, 0)),
                                        (2, ((dy + 1) * W + 1, W - 1))):
                                    ti = dy * 3 + dx
                                    col = (g * K * K + ti) * b_per + b
                                    in0 = (xt[:, off:off + rows_band * W]
                                           .rearrange("p (r c) -> p r c", c=W)
                                           [:, :, 0:1])
                                    oe = otv[:, :, oc:oc + 1]
                                    nc.vector.scalar_tensor_tensor(
                                        out=oe, in0=in0,
                                        scalar=wdn[:, col:col + 1], in1=oe,
                                        op0=AluOpType.mult, op1=AluOpType.add)

                            # output DMA on the ACT HWDGE ring
                            nc.scalar.dma_start(
                                y_d.ap()[b, 128 * g:128 * (g + 1),
                                         r0:r0 + rows_band, :],
                                ot[:])

    nc.compile()
    return nc


def make_in_maps(x, z, W_lin, b_per=B_PER):
    """Host-side shard + layout transforms (no math)."""
    wl = np.asarray(W_lin, dtype=np.float32)
    wlperm = (wl.reshape(G, 128, K * K, Z_DIM)
                .transpose(0, 2, 1, 3)
                .reshape(OUT_C * K * K, Z_DIM))
    wlt = np.ascontiguousarray(wlperm.T)                  # [64, 2304]
    ident = np.eye(128, dtype=np.float32)
    x = np.asarray(x, dtype=np.float32)
    z = np.asarray(z, dtype=np.float32)
    in_maps = []
    for c in range(N_CORES):
        sl = slice(c * b_per, (c + 1) * b_per)
        in_maps.append({
            "x": np.ascontiguousarray(x[sl]),
            "zT": np.ascontiguousarray(z[sl].T),          # [64, b_per]
            "wlt": wlt,
            "ident": ident,
            "zeros": np.zeros((128, W), dtype=np.float32),
        })
    return in_maps


_NC_CACHE = {}


def kernel(x, z, W_lin):
    key = "main"
    if key not in _NC_CACHE:
        _NC_CACHE[key] = build_nc()
    nc = _NC_CACHE[key]
    in_maps = make_in_maps(x, z, W_lin)
    res = bass_utils.run_bass_kernel_spmd(nc, in_maps, core_ids=list(range(N_CORES)))
    out = np.concatenate([res.results[c]["y"] for c in range(N_CORES)], axis=0)
    return out.astype(np.float32, copy=False)


# revision 21
# speedup vs baseline: 1.0170x; 1.0170x over previous
"""Trainium2 Bass kernel: per-sample hypernetwork depthwise 3x3 conv.

Reference computation (per batch b):
    W_dw[b] = (z[b] @ W_lin.T).reshape(OUT_C, 1, 3, 3)
    y[b]    = depthwise_conv2d(x[b], W_dw[b], padding=1)

Sharding: data-parallel over batch across 8 NeuronCores (2 batches/core),
W_lin replicated. Each core computes its own W_dw on-device.

Per-core design:
  - channels (256) -> 2 groups of 128 on SBUF partitions
  - image split into 32-row bands; each band is DMA'd as one fully
    contiguous 17.4KB-per-partition transfer into an UNPADDED flat tile
    (rows r0-1..r0+32 back to back, one pad element at each end).
    Width-edge taps wrap into the neighboring row; those wrong
    contributions are subtracted afterwards by small "correction" ops
    with negated weights reading the exact same wrapped positions.
  - 9 conv taps split between engines per 16-row PSUM group, alternating
    7-PE/2-DVE and 6-PE/3-DVE:
      * PE taps: diagonal-weight matmuls (float32r) accumulating in PSUM
      * DVE taps: fused scalar_tensor_tensor FMAs; the first also drains
        PSUM
  - ScalarE (ACT) makes the f32r rounded copy of each band and issues the
    output DMAs (separate HWDGE ring from the input DMAs on SyncE)
  - GpSimd only does the small halo memsets
  - W_dw computed on-device by 18 small fp32 matmuls from a host-side
    re-layout of W_lin (pure permutation/transpose, no host math)
"""

import os
import sys

for _p in ("/opt/trn_rl_repo", "/root/.axon_site", "/root/.axon_site/_ro/trn_rl_repo",
           "/root/.axon_site/_ro/pypackages"):
    if os.path.isdir(_p) and _p not in sys.path:
        sys.path.append(_p)

import numpy as np

import concourse.bass as bass
import concourse.tile as tile
from concourse import bacc, mybir
from concourse import bass_utils
from concourse.alu_op_type import AluOpType

F32 = mybir.dt.float32
F32R = mybir.dt.float32r

# problem constants (hardcoded per contract)
B, OUT_C, H, W = 16, 256, 128, 128
K, Z_DIM = 3, 64
N_CORES = 8
B_PER = B // N_CORES          # 2 batches per core
G = OUT_C // 128              # 2 channel groups of 128

# tuning knobs
PE_TAP_PATTERN = (7, 6)       # taps on PE per psum-group, alternating
ROWS_BAND = 32
ROWS_GROUP = 16
MAX_PE_TAPS = max(PE_TAP_PATTERN)

TAPS = [(dy, dx) for dy in range(3) for dx in range(3)]


def build_nc(pe_pattern=PE_TAP_PATTERN, rows_band=ROWS_BAND, rows_group=ROWS_GROUP,
             b_per=B_PER, h=H):
    """Build the per-core Bass program. Returns compiled Bacc object."""
    n_bands = h // rows_band
    grp_per_band = rows_band // rows_group
    banks_per_grp = rows_group * W // 512
    grp_free = rows_group * W
    band_free = rows_band * W
    pad_rows = rows_band + 2
    # data at flat [1 + t*W, 1 + (t+1)*W); leading pad at 0, trailing wrap pad
    # at pad_rows*W + 1; extra W slack so stride-W correction APs stay in range
    flat_n = pad_rows * W + 2 + W
    max_pe = 9  # final band runs all taps on PE

    nc = bacc.Bacc("TRN2", target_bir_lowering=False, debug=False)

    x_d = nc.dram_tensor("x", [b_per, OUT_C, h, W], F32, kind="ExternalInput")
    zt_d = nc.dram_tensor("zT", [Z_DIM, b_per], F32, kind="ExternalInput")
    wlt_d = nc.dram_tensor("wlt", [Z_DIM, OUT_C * K * K], F32, kind="ExternalInput")
    ident_d = nc.dram_tensor("ident", [128, 128], F32, kind="ExternalInput")
    zeros_d = nc.dram_tensor("zeros", [128, W], F32, kind="ExternalInput")
    y_d = nc.dram_tensor("y", [b_per, OUT_C, h, W], F32, kind="ExternalOutput")

    n_chunks = OUT_C * K * K // 128          # 18
    wd_cols = K * K * G * b_per              # 36, col = (g*9 + t)*b_per + b

    with tile.TileContext(nc) as tc:
        with tc.tile_pool(name="wconst", bufs=1) as wpool:
            ident = wpool.tile([128, 128], F32)
            nc.sync.dma_start(ident[:], ident_d.ap()[:, :])
            wlt = wpool.tile([Z_DIM, OUT_C * K * K], F32)
            half = OUT_C * K * K // 2
            nc.sync.dma_start(wlt[:, 0:half], wlt_d.ap()[:, 0:half])
            nc.sync.dma_start(wlt[:, half:], wlt_d.ap()[:, half:])
            zt = wpool.tile([Z_DIM, b_per], F32)
            nc.sync.dma_start(zt[:], zt_d.ap()[:, :])

            wd = wpool.tile([128, wd_cols], F32)
            with tc.tile_pool(name="wpsum", bufs=2, space="PSUM") as wps:
                for j in range(n_chunks):
                    ps = wps.tile([128, b_per], F32)
                    nc.tensor.matmul(ps[:], wlt[:, 128 * j:128 * (j + 1)], zt[:],
                                     start=True, stop=True)
                    nc.scalar.copy(wd[:, b_per * j:b_per * (j + 1)], ps[:])

            # negated weights for the wrap corrections
            wdn = wpool.tile([128, wd_cols], F32)
            nc.scalar.mul(wdn[:], wd[:], -1.0)

            # diagonal weight matrices (f32r) for all taps that may run on PE
            diags = {}
            for b in range(b_per):
                for g in range(G):
                    for ti in range(max_pe):
                        col = (g * K * K + ti) * b_per + b
                        dtile = wpool.tile([128, 128], F32R, tag=f"diag_{b}_{g}_{ti}")
                        nc.scalar.mul(dtile[:], ident[:], wd[:, col:col + 1])
                        diags[(b, g, ti)] = dtile

            with tc.tile_pool(name="xband", bufs=6) as xpool, \
                 tc.tile_pool(name="oband", bufs=4) as opool, \
                 tc.tile_pool(name="psum", bufs=2, space="PSUM") as pspool:
                gidx = 0
                band_no = 0
                for b in range(b_per):
                    for g in range(G):
                        prev_xt = None
                        for band in range(n_bands):
                            r0 = band * rows_band
                            # rows r0-1, r0 come from the previous band's tile
                            # (SBUF->SBUF); only new rows come from HBM
                            lo = 0 if band == 0 else r0 + 1
                            hi = min(r0 + rows_band + 1, h)
                            # tile row t holds image row r0-1+t at flat
                            # [1 + t*W, 1 + (t+1)*W)
                            xt = xpool.tile([128, flat_n], F32)
                            xtr = xt[:].bitcast(F32R)
                            # zero wrap-pad elements once per pool buffer; later
                            # reuses hold stale-but-finite values that cancel
                            # exactly against the corrections
                            if band_no < 6:
                                for off in (0, pad_rows * W + 1):
                                    nc.vector.tensor_scalar(
                                        out=xtr[:, off:off + 1], in0=ident[:, 0:1],
                                        scalar1=0.0, scalar2=None, op0=AluOpType.mult)
                            band_no += 1
                            if r0 == 0:
                                nc.scalar.dma_start(xtr[:, 1:1 + W],
                                                    zeros_d.ap()[:, :].bitcast(F32R))
                            if r0 + rows_band == h:
                                nc.scalar.dma_start(
                                    xtr[:, 1 + (pad_rows - 1) * W:1 + pad_rows * W],
                                    zeros_d.ap()[:, :].bitcast(F32R))
                            # dest AP typed f32r: PE reads it as f32r (the HW
                            # rounds on read); DVE reads the same bits as f32
                            if band > 0:
                                nc.sync.dma_start(
                                    xtr[:, 1:1 + 2 * W],
                                    prev_xt[:, 1 + (rows_band) * W:
                                            1 + (rows_band + 2) * W].bitcast(F32R))
                            dst0 = 1 + (lo - (r0 - 1)) * W
                            nc.sync.dma_start(
                                xt[:, dst0:dst0 + (hi - lo) * W].bitcast(F32R),
                                x_d.ap()[b, 128 * g:128 * (g + 1), lo:hi, :]
                                .bitcast(F32R))
                            prev_xt = xt[:]
                            xr = xt[:].bitcast(F32R)

                            ot = opool.tile([128, band_free], F32)
                            last_band = (b == b_per - 1 and g == G - 1
                                         and band == n_bands - 1)
                            for grp in range(grp_per_band):
                                gr0 = grp * rows_group
                                n_pe = 9 if last_band else \
                                    pe_pattern[gidx % len(pe_pattern)]
                                gidx += 1
                                pe_taps = TAPS[:n_pe]
                                dve_taps = TAPS[n_pe:]

                                ps = pspool.tile([128, grp_free], F32)
                                rows_bank = 512 // W
                                for ti in range(n_pe):
                                    dy, dx = pe_taps[ti]
                                    for bank in range(banks_per_grp):
                                        s = (gr0 + bank * rows_bank + dy) * W + dx
                                        nc.tensor.matmul(
                                            ps[:, 512 * bank:512 * (bank + 1)],
                                            diags[(b, g, ti)][:],
                                            xr[:, s:s + 512],
                                            start=(ti == 0),
                                            stop=(ti == n_pe - 1))

                                og = ot[:, gr0 * W:gr0 * W + grp_free]
                                acc = ps[:]
                                for k, (dy, dx) in enumerate(dve_taps):
                                    ti = n_pe + k
                                    col = (g * K * K + ti) * b_per + b
                                    s = (gr0 + dy) * W + dx
                                    nc.vector.scalar_tensor_tensor(
                                        out=og, in0=xt[:, s:s + grp_free],
                                        scalar=wd[:, col:col + 1], in1=acc,
                                        op0=AluOpType.mult, op1=AluOpType.add)
                                    acc = og
                                if not dve_taps:
                                    nc.vector.tensor_copy(og, ps[:])

                            # width-edge wrap corrections over the whole band:
                            # og[r, 0]   -= w[dy,0] * flat[(r+dy)*W]      (left)
                            # og[r, W-1] -= w[dy,2] * flat[(r+dy+1)*W+1]  (right)
                            otv = ot[:].rearrange("p (r c) -> p r c", c=W)
                            for dy in range(3):
                                for dx, (off, oc) in (
                                        (0, (dy * W, 0)),
                                        (2, ((dy + 1) * W + 1, W - 1))):
                                    ti = dy * 3 + dx
                                    col = (g * K * K + ti) * b_per + b
                                    in0 = (xt[:, off:off + rows_band * W]
                                           .rearrange("p (r c) -> p r c", c=W)
                                           [:, :, 0:1])
                                    oe = otv[:, :, oc:oc + 1]
                                    nc.vector.scalar_tensor_tensor(
                                        out=oe, in0=in0,
                                        scalar=wdn[:, col:col + 1], in1=oe,
                                        op0=AluOpType.mult, op1=AluOpType.add)

                            # output DMA on the ACT HWDGE ring
                            nc.scalar.dma_start(
                                y_d.ap()[b, 128 * g:128 * (g + 1),
                                         r0:r0 + rows_band, :],
                                ot[:])

    nc.compile()
    return nc


def make_in_maps(x, z, W_lin, b_per=B_PER):
    """Host-side shard + layout transforms (no math)."""
    wl = np.asarray(W_lin, dtype=np.float32)
    wlperm = (wl.reshape(G, 128, K * K, Z_DIM)
                .transpose(0, 2, 1, 3)
                .reshape(OUT_C * K * K, Z_DIM))
    wlt = np.ascontiguousarray(wlperm.T)                  # [64, 2304]
    ident = np.eye(128, dtype=np.float32)
    x = np.asarray(x, dtype=np.float32)
    z = np.asarray(z, dtype=np.float32)
    in_maps = []
    for c in range(N_CORES):
        sl = slice(c * b_per, (c + 1) * b_per)
        in_maps.append({
            "x": np.ascontiguousarray(x[sl]),
            "zT": np.ascontiguousarray(z[sl].T),          # [64, b_per]
            "wlt": wlt,
            "ident": ident,
            "zeros": np.zeros((128, W), dtype=np.float32),
        })
    return in_maps


_NC_CACHE = {}


def kernel(x, z, W_lin):
    key = "main"
    if key not in _NC_CACHE:
        _NC_CACHE[key] = build_nc()
    nc = _NC_CACHE[key]
    in_maps = make_in_maps(x, z, W_lin)
    res = bass_utils.run_bass_kernel_spmd(nc, in_maps, core_ids=list(range(N_CORES)))
    out = np.concatenate([res.results[c]["y"] for c in range(N_CORES)], axis=0)
    return out.astype(np.float32, copy=False)
